# revision 34
# baseline (speedup 1.0000x reference)
"""Trainium2 Bass kernel for causal multi-head attention with RoPE.

Problem: B=1, S=4096, D=1024, H=16 heads of HD=64.
  q/k/v = x @ w{q,k,v}.T ; rope(q), rope(k); scores = q k^T/sqrt(HD) + mask;
  out = softmax(scores) @ v ; y = out @ wo.T

Sharding: tensor-parallel over heads. 8 cores x 2 heads each.  Each core
computes its 2 heads' q/k/v projections (column-split weights), full
attention for those heads over all 4096 positions, and a partial output
projection (row-split wo).  The host sums the 8 partial [S, D] outputs
(device writes bf16 partials; host accumulates in fp32).

v3 design notes:
  - Scores for BOTH heads of one sk-chunk go into ONE [128, 1024] PSUM
    tile (2 banks) and are exp'd by ONE ScalarE ACTIVATE (the kernel's
    per-instruction ACT overhead is 352 cycles; batching halves it).
  - Diagonal (partially masked) chunks trim the exp and the p@v matmul
    by whole 128-col sub-blocks instead of zero-filling probabilities.
  - q/k (post-rope), probabilities and v are bf16: scores here are |s|<4
    so precision is ample (~1e-3 final); bf16 weights get fast-weight-load
    on the PE and lighter DVE traffic.
  - The softmax epilogue is software-pipelined by one q-block: after the
    last p@v matmul of block J the unnormalized [65,512] accumulators
    (attn rows + denominator row 64) are copied to SBUF, releasing the
    PSUM slots immediately; the denominator reciprocal chain, the output
    projection and the y store for block J run while block J+1's scores
    and exp proceed.  This keeps the PE dense enough that the HAM clock
    gate stays at full rate.
  - PSUM budget (8 banks): sc 2x[128,1024] (4) + pv 2x[128,512] (2) +
    aux 2x[128,512] (2) where aux multiplexes proj/rope/transpose/outproj.
  - Projections for s-block J+1 are emitted behind attention for block J.
"""

import os
import sys

import numpy as np

sys.path.insert(0, "/opt/trn_rl_repo")

S = 4096
D = 1024
H = 16
HD = 64
NCORES = 8
HPC = H // NCORES  # 2 heads per core
EC = HPC * HD  # 128 head-dim columns per core
NEG_THRESH = -1e8  # blocks entirely <= this are "fully masked"

_PROGRAM_CACHE = {}


def classify_mask(maskT, s):
    """Classify 128x128 blocks of mask.T: 0=zero, 1=neginf, 2=general."""
    nb = s // 128
    cls = np.zeros((nb, nb), np.int8)
    for i in range(nb):
        for j in range(nb):
            blk = maskT[128 * i : 128 * (i + 1), 128 * j : 128 * (j + 1)]
            if np.all(blk == 0.0):
                cls[i, j] = 0
            elif np.all(blk <= NEG_THRESH):
                cls[i, j] = 1
            else:
                cls[i, j] = 2
    return cls


def build_program(s, cls, n_gen, neg_bias):
    """Build the SPMD Bass/Tile program for one core (same for all cores)."""
    from contextlib import ExitStack

    import concourse.bass as bass
    import concourse.tile as tile
    from concourse import bacc, mybir

    f32 = mybir.dt.float32
    f32r = mybir.dt.float32r
    bf16 = mybir.dt.bfloat16
    Exp = mybir.ActivationFunctionType.Exp

    nb = s // 128  # sk chunks
    NJ = s // 512  # sq blocks
    nd = D // 128  # contraction chunks for projections

    nc = bacc.Bacc(
        "TRN2", target_bir_lowering=False, debug=False, num_devices=NCORES
    )

    xT = nc.dram_tensor("xT", [D, s], bf16, kind="ExternalInput").ap()
    cosT = nc.dram_tensor("cosT", [128, s], f32, kind="ExternalInput").ap()
    sinTS = nc.dram_tensor("sinTS", [128, s], f32, kind="ExternalInput").ap()
    wqT = nc.dram_tensor("wqT", [128, D], bf16, kind="ExternalInput").ap()
    wkT = nc.dram_tensor("wkT", [128, D], bf16, kind="ExternalInput").ap()
    wvT = nc.dram_tensor("wvT", [128, D], bf16, kind="ExternalInput").ap()
    woT = nc.dram_tensor("woT", [128, D], f32r, kind="ExternalInput").ap()
    P128 = nc.dram_tensor("P128", [128, 128], f32r, kind="ExternalInput").ap()
    I128 = nc.dram_tensor("I128", [128, 128], f32, kind="ExternalInput").ap()
    maskg = nc.dram_tensor(
        "maskg", [128, 128 * max(n_gen, 1)], f32, kind="ExternalInput"
    ).ap()
    ones2 = nc.dram_tensor("ones2", [128, 130], bf16, kind="ExternalInput").ap()
    y = nc.dram_tensor("y", [s, D], bf16, kind="ExternalOutput").ap()

    r = lambda ap: ap

    with tile.TileContext(nc) as tc, ExitStack() as ctx:
        consts = ctx.enter_context(tc.tile_pool(name="consts", bufs=1))
        persist = ctx.enter_context(tc.tile_pool(name="persist", bufs=1))
        xt_pool = ctx.enter_context(tc.tile_pool(name="xt", bufs=24))
        wk_pool = ctx.enter_context(tc.tile_pool(name="work", bufs=4))
        probs_pool = ctx.enter_context(tc.tile_pool(name="probs", bufs=8))
        mask_pool = ctx.enter_context(tc.tile_pool(name="maskb", bufs=2))
        atu_pool = ctx.enter_context(tc.tile_pool(name="atu", bufs=4))
        attn_pool = ctx.enter_context(tc.tile_pool(name="attn", bufs=2))
        bc_pool = ctx.enter_context(tc.tile_pool(name="bc", bufs=4))
        yo_pool = ctx.enter_context(tc.tile_pool(name="yo", bufs=3))
        psum = ctx.enter_context(tc.tile_pool(name="psum", bufs=2, space="PSUM"))

        # ---- constants ----
        # DMA throughput is packet-rate bound (one packet per partition
        # row), so big constants are split by partition halves across
        # otherwise-idle engine queues to parallelize descriptor issue.
        c_wq = consts.tile([128, D], bf16)
        nc.sync.dma_start(c_wq[0:64, :], wqT[0:64, :])
        nc.scalar.dma_start(c_wq[64:128, :], wqT[64:128, :])
        c_wk = consts.tile([128, D], bf16)
        nc.sync.dma_start(c_wk[0:64, :], wkT[0:64, :])
        nc.scalar.dma_start(c_wk[64:128, :], wkT[64:128, :])
        c_wv = consts.tile([128, D], bf16)
        nc.sync.dma_start(c_wv[0:64, :], wvT[0:64, :])
        nc.scalar.dma_start(c_wv[64:128, :], wvT[64:128, :])
        c_P = consts.tile([128, 128], f32r)
        nc.gpsimd.dma_start(c_P[:], P128[:])
        c_I = consts.tile([128, 128], f32)
        nc.gpsimd.dma_start(c_I[:], I128[:])
        # cos/sin are streamed per s-block inside proj(); wo/ones after proj(0)
        c_cos = consts.tile([128, s], f32)
        c_sin = consts.tile([128, s], f32)
        c_one2 = consts.tile([128, 130], bf16)  # cols 0:2 ones, 2:130 zeros
        c_wo = consts.tile([128, D], f32r)

        # pre-load the exp table set while projections run
        warm = wk_pool.tile([1, 2], f32, tag="warm", bufs=1)
        nc.scalar.activation(warm[:], c_I[0:1, 0:2], Exp)

        # ---- persistent activations ----
        qT2 = persist.tile([128, s], bf16)  # [2*64 head rows, s] rope'd & scaled
        kT2 = persist.tile([128, s], bf16)
        v_sb = persist.tile([128, nb * 130], bf16)  # per sk-chunk: [A 64|1|B 64|1]

        # ---- phase-1 worker: q/k/v projections + rope for one 512 s-block ----
        # Returns a list of closures ("steps") so the main loop can weave
        # them between attention chunks: each engine executes its FIFO in
        # emission order, so projection matmuls emitted between score
        # matmuls fill the PE while ScalarE crunches exp.
        def proj_steps(sb):
            ssl = slice(512 * sb, 512 * (sb + 1))
            st = {}

            def s_head():
                # issue every DMA for this block up front: they ride the
                # sync/gpsimd/scalar queues, so by the time the woven matmul
                # steps reach the PE FIFO the tiles have landed
                nc.scalar.dma_start(c_cos[:, ssl], cosT[:, ssl])
                nc.gpsimd.dma_start(c_sin[:, ssl], sinTS[:, ssl])
                st["xts"] = []
                for dc in range(nd):
                    xt = xt_pool.tile([128, 512], bf16)
                    if sb == 0:
                        eng = (nc.sync, nc.gpsimd, nc.scalar)[dc % 3]
                    else:
                        eng = nc.sync if dc % 2 == 0 else nc.gpsimd
                    eng.dma_start(xt[:], xT[128 * dc : 128 * (dc + 1), ssl])
                    st["xts"].append(xt)
                st["psq"] = psum.tile([128, 512], f32, tag="aux", name="psq")
                st["psk"] = psum.tile([128, 512], f32, tag="aux", name="psk")

            def s_qk(dc):
                def f():
                    xt = st["xts"][dc]
                    first, last = dc == 0, dc == nd - 1
                    nc.tensor.matmul(
                        st["psq"][:], r(c_wq[:, 128 * dc : 128 * (dc + 1)]),
                        r(xt[:]), start=first, stop=last, skip_group_check=True,
                    )
                    nc.tensor.matmul(
                        st["psk"][:], r(c_wk[:, 128 * dc : 128 * (dc + 1)]),
                        r(xt[:]), start=first, stop=last, skip_group_check=True,
                    )
                return f

            def s_rope(which):
                def f():
                    ps = st["psq"] if which == "q" else st["psk"]
                    dst = qT2 if which == "q" else kT2
                    raw = wk_pool.tile([128, 512], f32r, tag="rope")
                    nc.vector.tensor_copy(raw[:], ps[:])
                    psw = psum.tile([128, 512], f32, tag="aux")
                    nc.tensor.matmul(
                        psw[:], r(c_P[:]), r(raw[:]), start=True, stop=True
                    )
                    t1 = wk_pool.tile([128, 512], f32, tag="rope")
                    nc.vector.tensor_mul(t1[:], raw[:], c_cos[:, ssl])
                    t2 = wk_pool.tile([128, 512], f32, tag="rope")
                    nc.vector.tensor_mul(t2[:], psw[:], c_sin[:, ssl])
                    nc.vector.tensor_add(dst[:, ssl], t1[:], t2[:])
                return f

            def s_vhead():
                st["psv"] = psum.tile([128, 512], f32, tag="aux", name="psv")

            def s_v(dc):
                def f():
                    nc.tensor.matmul(
                        st["psv"][:], r(c_wv[:, 128 * dc : 128 * (dc + 1)]),
                        r(st["xts"][dc][:]), start=dc == 0, stop=dc == nd - 1,
                        skip_group_check=True,
                    )
                return f

            def s_vtt():
                vtt = wk_pool.tile([128, 512], f32, tag="vtt", bufs=2)
                nc.vector.tensor_copy(vtt[:], st["psv"][:])
                st["vtt"] = vtt

            def s_tr(k4):
                def f():
                    sc_ = 4 * sb + k4
                    pst = psum.tile([128, 512], f32, tag="aux")
                    nc.tensor.transpose(
                        pst[:, 0:128], st["vtt"][:, 128 * k4 : 128 * (k4 + 1)],
                        c_I[:],
                    )
                    nc.vector.tensor_copy(
                        v_sb[:, 130 * sc_ : 130 * sc_ + 64], pst[:, 0:64]
                    )
                    nc.vector.tensor_copy(
                        v_sb[:, 130 * sc_ + 65 : 130 * sc_ + 129], pst[:, 64:128]
                    )
                return f

            steps = [s_head]
            steps += [s_qk(dc) for dc in range(nd)]
            steps += [s_rope("q"), s_rope("k"), s_vhead]
            steps += [s_v(dc) for dc in range(nd)]
            steps += [s_vtt]
            steps += [s_tr(k4) for k4 in range(4)]
            return steps

        def proj(sb):
            for f in proj_steps(sb):
                f()

        def kept_for(J):
            kept = []
            for i in range(nb):
                subs = [int(cls[i, 4 * J + u]) for u in range(4)]
                if any(c != 1 for c in subs):
                    kept.append((i, subs))
            return kept

        # ---- attention core for one q block: scores -> exp -> p@v ----
        # leaves unnormalized [65, 512] accumulators (+den in row 64) in SBUF.
        # `fillers` are closures (projection / epilogue steps) emitted between
        # chunks so the PE FIFO always has ready work while exp runs; the p@v
        # matmuls are emitted one chunk late so they never head-of-line block.
        def attn_core(J, kept, fillers=()):
            jsl = slice(512 * J, 512 * (J + 1))
            # batched DMA of the general (mixed) mask blocks for this J
            gen = [(i, u) for i, subs in kept for u in range(4) if subs[u] == 2]
            mb_slc = {}
            if gen:
                gis = sorted(GEN_INDEX[(i, 4 * J + u)] for i, u in gen)
                runs = []
                for g in gis:
                    if runs and g == runs[-1][1]:
                        runs[-1][1] = g + 1
                    else:
                        runs.append([g, g + 1])
                for g0, g1 in runs:
                    mb = mask_pool.tile([128, 512], f32)
                    nc.gpsimd.dma_start(
                        mb[:, : 128 * (g1 - g0)],
                        maskg[:, 128 * g0 : 128 * g1],
                    )
                    for g in range(g0, g1):
                        mb_slc[g] = mb[:, 128 * (g - g0) : 128 * (g - g0 + 1)]
            pvA = psum.tile([128, 512], f32, tag="pv")
            pvB = psum.tile([128, 512], f32, tag="pv")
            n = len(kept)
            fillers = list(fillers)
            fi = 0

            def emit_pv(idx):
                i, subs = kept[idx]
                fk = min(u for u in range(4) if subs[u] != 1)
                pb = pbs[idx]
                first, last = idx == 0, idx == n - 1
                tsl = slice(128 * fk, 512)
                nc.tensor.matmul(
                    pvA[0:65, tsl], r(v_sb[:, 130 * i : 130 * i + 65]),
                    r(pb[:, 128 * fk : 512]),
                    start=first, stop=last, skip_group_check=True,
                )
                nc.tensor.matmul(
                    pvB[0:65, tsl], r(v_sb[:, 130 * i + 65 : 130 * i + 130]),
                    r(pb[:, 512 + 128 * fk : 1024]),
                    start=first, stop=last, skip_group_check=True,
                )

            pbs = []
            for idx, (i, subs) in enumerate(kept):
                isl = slice(128 * i, 128 * (i + 1))
                fk = min(u for u in range(4) if subs[u] != 1)
                sc = psum.tile([128, 1024], f32, tag="sc")
                nc.tensor.matmul(
                    sc[:, 0:512], r(kT2[0:64, isl]), r(qT2[0:64, jsl]),
                    start=True, stop=True, tile_position=(0, 0),
                )
                nc.tensor.matmul(
                    sc[:, 512:1024], r(kT2[64:128, isl]), r(qT2[64:128, jsl]),
                    start=True, stop=True, tile_position=(64, 0),
                )
                sc3v = sc[:].rearrange("p (h q) -> p h q", h=2)
                for u, cu in enumerate(subs):
                    if cu == 2:
                        mbs = mb_slc[GEN_INDEX[(i, 4 * J + u)]]
                        s3 = sc3v[:, :, 128 * u : 128 * (u + 1)]
                        nc.vector.tensor_add(
                            s3, s3, mbs[:, None, :].broadcast_to([128, 2, 128])
                        )
                pb = probs_pool.tile([128, 1024], bf16)
                pbs.append(pb)
                if fk:
                    sc3 = sc[:].rearrange("p (h q) -> p h q", h=2)[:, :, 128 * fk : 512]
                    pb3 = pb[:].rearrange("p (h q) -> p h q", h=2)[:, :, 128 * fk : 512]
                    nc.scalar.activation(pb3, sc3, Exp, bias=-neg_bias)
                else:
                    nc.scalar.activation(pb[:], sc[:], Exp, bias=-neg_bias)
                # middle fully-masked sub-blocks (never happens for causal):
                for u, cu in enumerate(subs):
                    if cu == 1 and u > fk:
                        for off in (0, 512):
                            usl = slice(off + 128 * u, off + 128 * (u + 1))
                            nc.vector.tensor_copy(pb[:, usl], c_one2[:, 2:130])
                # weave in filler work, then the previous chunk's p@v
                want = (idx + 1) * len(fillers) // n
                while fi < want:
                    fillers[fi]()
                    fi += 1
                if idx > 0:
                    emit_pv(idx - 1)
            while fi < len(fillers):
                fillers[fi]()
                fi += 1
            emit_pv(n - 1)
            # evacuate unnormalized accumulators; releases the pv PSUM slots
            atus = []
            for pv in (pvA, pvB):
                atu = atu_pool.tile([65, 512], f32)
                nc.vector.tensor_copy(atu[:], pv[0:65, :])
                atus.append(atu)
            return atus

        # ---- epilogue for one q block: normalize, project, store ----
        # Split into [head..., tail...] step closures: the reciprocal chain
        # (DVE/DMA/gpsimd, long latency, no PE) goes early in the weave, the
        # output-projection matmuls late (by then `at` is ready).
        def epilogue_steps(J, atus):
            atuA, atuB = atus
            st = {"bcs": []}

            def e_den(atu):
                def f():
                    d0 = wk_pool.tile([1, 512], f32, tag="d0", bufs=8)
                    nc.gpsimd.dma_start(d0[:], atu[64:65, :])
                    r0 = wk_pool.tile([1, 512], f32, tag="d0", bufs=8)
                    s0 = wk_pool.tile([1, 512], f32, tag="d0", bufs=8)
                    nc.vector.reciprocal_approx_accurate(r0[:], d0[:], s0[:])
                    bc = bc_pool.tile([64, 512], f32)
                    nc.gpsimd.partition_broadcast(bc[:], r0[:])
                    st["bcs"].append(bc)
                return f

            def e_at():
                at = attn_pool.tile([128, 512], f32r)
                atBt = attn_pool.tile([64, 512], f32r, tag="atB", bufs=2)
                nc.vector.tensor_mul(at[0:64, :], atuA[0:64, :], st["bcs"][0][:])
                nc.vector.tensor_mul(atBt[:], atuB[0:64, :], st["bcs"][1][:])
                nc.gpsimd.dma_start(at[64:128, :], atBt[:])
                st["at"] = at

            def e_out(scn):
                def f():
                    at = st["at"]
                    ssl2 = slice(128 * scn, 128 * (scn + 1))
                    rows = slice(512 * J + 128 * scn, 512 * J + 128 * (scn + 1))
                    yo = yo_pool.tile([128, 1024], bf16)
                    for do2 in range(2):
                        dsl = slice(512 * do2, 512 * (do2 + 1))
                        pso = psum.tile([128, 512], f32, tag="aux")
                        nc.tensor.matmul(
                            pso[:], r(at[:, ssl2]), r(c_wo[:, dsl]),
                            start=True, stop=True,
                        )
                        nc.vector.tensor_copy(yo[:, dsl], pso[:])
                    nc.sync.dma_start(y[rows, :], yo[:])
                return f

            head = [e_den(atuA), e_den(atuB), e_at]
            tail = [e_out(scn) for scn in range(4)]
            return head, tail

        def epilogue(J, atus):
            head, tail = epilogue_steps(J, atus)
            for f in head + tail:
                f()

        # ---- main schedule: 1-block software pipeline ----
        # iteration J: [epilogue J-1 dataflow] + [attn core J] + [proj J+1]
        kept_all = [kept_for(J) for J in range(NJ)]
        done_sb = 0
        pending = None  # (J, atus) awaiting epilogue
        first = True
        for J in range(NJ):
            need_sb = J
            if kept_all[J]:
                need_sb = max(need_sb, max(i for i, _ in kept_all[J]) // 4)
            while done_sb <= need_sb:
                proj(done_sb)
                done_sb += 1
            if first:
                # late-needed constants, deferred so they don't crowd the
                # startup DMA stream feeding the first projections
                nc.gpsimd.dma_start(c_one2[:], ones2[:])
                nc.gpsimd.dma_start(c_wo[:], woT[:])
                # ones columns of v_aug (cols 64/129 of each 130-wide chunk)
                ones_view = v_sb[:].rearrange("p (c w) -> p c w", w=130)[
                    :, :, 64:130:65
                ]
                nc.vector.tensor_copy(
                    ones_view, c_one2[:, None, 0:2].broadcast_to([128, nb, 2])
                )
                first = False
            fillers = []
            late = []
            if pending is not None:
                head, tail = epilogue_steps(*pending)
                fillers += head
                late = tail
                pending = None
            if J + 1 < NJ and done_sb == J + 1 and kept_all[J]:
                fillers += proj_steps(done_sb)
                done_sb += 1
            fillers += late
            if kept_all[J]:
                atus = attn_core(J, kept_all[J], fillers)
                pending = (J, atus)
            else:
                for f in fillers:
                    f()
        if pending is not None:
            epilogue(*pending)

    nc.compile()
    return nc


GEN_INDEX = {}


def host_prep(x, freqs_cos, freqs_sin, mask, wq, wk, wv, wo):
    """Build per-core input maps + mask classification.  Returns
    (in_maps, cls, n_gen, neg_bias)."""
    global GEN_INDEX
    s = x.shape[1]
    import ml_dtypes

    x2 = np.ascontiguousarray(x.reshape(s, D))
    xT = np.ascontiguousarray(x2.T).astype(ml_dtypes.bfloat16)

    # rope tables in T layout (same for q and k; q scale folded into wq)
    p = np.arange(128)
    j = (p % HD) // 2  # freq index per partition row
    cosT = np.ascontiguousarray(freqs_cos.T[j, :]).astype(np.float32)  # [128, s]
    sinT = freqs_sin.T[j, :]
    sign = np.where(p % 2 == 0, -1.0, 1.0).astype(np.float32)
    sinTS = np.ascontiguousarray(sinT * sign[:, None]).astype(np.float32)

    # swap-adjacent permutation and identity
    P = np.zeros((128, 128), np.float32)
    P[np.arange(128) ^ 1, np.arange(128)] = 1.0
    I = np.eye(128, dtype=np.float32)

    # mask classification + general block packing
    maskT = np.ascontiguousarray(mask.T).astype(np.float32)
    cls = classify_mask(maskT, s)
    GEN_INDEX = {}
    gen_blocks = []
    nbk = s // 128
    for i in range(nbk):
        for jj in range(nbk):
            if cls[i, jj] == 2:
                GEN_INDEX[(i, jj)] = len(gen_blocks)
                gen_blocks.append(
                    maskT[128 * i : 128 * (i + 1), 128 * jj : 128 * (jj + 1)]
                )
    n_gen = len(gen_blocks)
    if n_gen:
        maskg = np.ascontiguousarray(np.concatenate(gen_blocks, axis=1))
    else:
        maskg = np.zeros((128, 128), np.float32)

    # softmax shift: cheap norm bound first (rope preserves pair norms);
    # only fall back to the exact per-head max if the bound exceeds 60.
    qf = x2 @ wq.T
    kf = x2 @ wk.T
    qn = kn = 0.0
    for h in range(H):
        hs = slice(HD * h, HD * (h + 1))
        qn = max(qn, float(np.linalg.norm(qf[:, hs], axis=1).max()))
        kn = max(kn, float(np.linalg.norm(kf[:, hs], axis=1).max()))
    m_bound = qn * kn / 8.0
    if m_bound <= 60.0:
        neg_bias = 0.0
    else:
        qh = _rope_np(qf, freqs_cos, freqs_sin)
        kh = _rope_np(kf, freqs_cos, freqs_sin)
        m = 0.0
        for h in range(H):
            qs = qh[:, HD * h : HD * (h + 1)]
            ks = kh[:, HD * h : HD * (h + 1)]
            m = max(m, float(np.abs(qs @ ks.T).max()) / 8.0)
        neg_bias = max(0.0, m - 60.0)

    def pack_w(w_slice):  # [EC, D] -> [128, D] chunked-transpose layout
        t = np.ascontiguousarray(w_slice.T)  # [D, EC=128]
        return np.ascontiguousarray(
            t.reshape(D // 128, 128, 128).transpose(1, 0, 2).reshape(128, D)
        ).astype(ml_dtypes.bfloat16)

    ones130 = np.zeros((128, 130), ml_dtypes.bfloat16)
    ones130[:, 0:2] = 1.0
    in_maps = []
    for c in range(NCORES):
        R = slice(EC * c, EC * (c + 1))
        woT_pack = np.ascontiguousarray(wo[:, R].T).astype(np.float32)  # [128, D]
        in_maps.append(
            {
                "xT": xT,
                "cosT": cosT,
                "sinTS": sinTS,
                "wqT": pack_w(wq[R] * 0.125),
                "wkT": pack_w(wk[R]),
                "wvT": pack_w(wv[R]),
                "woT": woT_pack,
                "P128": P,
                "I128": I,
                "maskg": maskg,
                "ones2": ones130,
            }
        )
    return in_maps, cls, n_gen, neg_bias


def _rope_np(t, cos, sin):
    s = t.shape[0]
    tr = t.reshape(s, H, HD // 2, 2)
    te, to = tr[..., 0], tr[..., 1]
    c = cos[:, None, :]
    sn = sin[:, None, :]
    oe = te * c - to * sn
    oo = te * sn + to * c
    return np.stack([oe, oo], axis=-1).reshape(s, H * HD)


def kernel(**inputs):
    from concourse.bass_utils import run_bass_kernel_spmd

    x = np.asarray(inputs["x"], np.float32)
    in_maps, cls, n_gen, neg_bias = host_prep(
        x,
        np.asarray(inputs["freqs_cos"], np.float32),
        np.asarray(inputs["freqs_sin"], np.float32),
        np.asarray(inputs["mask"], np.float32),
        np.asarray(inputs["wq"], np.float32),
        np.asarray(inputs["wk"], np.float32),
        np.asarray(inputs["wv"], np.float32),
        np.asarray(inputs["wo"], np.float32),
    )
    key = (x.shape[1], cls.tobytes(), n_gen, float(neg_bias))
    if key not in _PROGRAM_CACHE:
        _PROGRAM_CACHE[key] = build_program(x.shape[1], cls, n_gen, neg_bias)
    nc = _PROGRAM_CACHE[key]
    res = run_bass_kernel_spmd(nc, in_maps, core_ids=list(range(NCORES)))
    y = np.zeros((x.shape[1], D), np.float32)
    for c in range(NCORES):
        y += np.asarray(res.results[c]["y"], np.float32)
    return y.reshape(x.shape)


# revision 36
# speedup vs baseline: 1.1649x; 1.1649x over previous
"""Trainium2 Bass kernel for causal multi-head attention with RoPE.

Problem: B=1, S=4096, D=1024, H=16 heads of HD=64.
  q/k/v = x @ w{q,k,v}.T ; rope(q), rope(k); scores = q k^T/sqrt(HD) + mask;
  out = softmax(scores) @ v ; y = out @ wo.T

Sharding: tensor-parallel over heads. 8 cores x 2 heads each.  Each core
computes its 2 heads' q/k/v projections (column-split weights), full
attention for those heads over all 4096 positions, and a partial output
projection (row-split wo).  The host sums the 8 partial [S, D] outputs
(device writes bf16 partials; host accumulates in fp32).

v3 design notes:
  - Scores for BOTH heads of one sk-chunk go into ONE [128, 1024] PSUM
    tile (2 banks) and are exp'd by ONE ScalarE ACTIVATE (the kernel's
    per-instruction ACT overhead is 352 cycles; batching halves it).
  - Diagonal (partially masked) chunks trim the exp and the p@v matmul
    by whole 128-col sub-blocks instead of zero-filling probabilities.
  - q/k (post-rope), probabilities and v are bf16: scores here are |s|<4
    so precision is ample (~1e-3 final); bf16 weights get fast-weight-load
    on the PE and lighter DVE traffic.
  - The softmax epilogue is software-pipelined by one q-block: after the
    last p@v matmul of block J the unnormalized [65,512] accumulators
    (attn rows + denominator row 64) are copied to SBUF, releasing the
    PSUM slots immediately; the denominator reciprocal chain, the output
    projection and the y store for block J run while block J+1's scores
    and exp proceed.  This keeps the PE dense enough that the HAM clock
    gate stays at full rate.
  - PSUM budget (8 banks): sc 2x[128,1024] (4) + pv 2x[128,512] (2) +
    aux 2x[128,512] (2) where aux multiplexes proj/rope/transpose/outproj.
  - Projections for s-block J+1 are emitted behind attention for block J.
"""

import os
import sys

import numpy as np

sys.path.insert(0, "/opt/trn_rl_repo")

S = 4096
D = 1024
H = 16
HD = 64
NCORES = 8
HPC = H // NCORES  # 2 heads per core
EC = HPC * HD  # 128 head-dim columns per core
NEG_THRESH = -1e8  # blocks entirely <= this are "fully masked"

_PROGRAM_CACHE = {}


def classify_mask(maskT, s):
    """Classify 128x128 blocks of mask.T: 0=zero, 1=neginf, 2=general."""
    nb = s // 128
    cls = np.zeros((nb, nb), np.int8)
    for i in range(nb):
        for j in range(nb):
            blk = maskT[128 * i : 128 * (i + 1), 128 * j : 128 * (j + 1)]
            if np.all(blk == 0.0):
                cls[i, j] = 0
            elif np.all(blk <= NEG_THRESH):
                cls[i, j] = 1
            else:
                cls[i, j] = 2
    return cls


def build_program(s, cls, n_gen, neg_bias):
    """Build the SPMD Bass/Tile program for one core (same for all cores)."""
    from contextlib import ExitStack

    import concourse.bass as bass
    import concourse.tile as tile
    from concourse import bacc, mybir

    f32 = mybir.dt.float32
    f32r = mybir.dt.float32r
    bf16 = mybir.dt.bfloat16
    Exp = mybir.ActivationFunctionType.Exp

    nb = s // 128  # sk chunks
    NJ = s // 512  # sq blocks
    nd = D // 128  # contraction chunks for projections

    nc = bacc.Bacc(
        "TRN2", target_bir_lowering=False, debug=False, num_devices=NCORES
    )

    xT = nc.dram_tensor("xT", [D, s], bf16, kind="ExternalInput").ap()
    cosT = nc.dram_tensor("cosT", [128, s], f32, kind="ExternalInput").ap()
    sinTS = nc.dram_tensor("sinTS", [128, s], f32, kind="ExternalInput").ap()
    wqT = nc.dram_tensor("wqT", [128, D], bf16, kind="ExternalInput").ap()
    wkT = nc.dram_tensor("wkT", [128, D], bf16, kind="ExternalInput").ap()
    wvT = nc.dram_tensor("wvT", [128, D], bf16, kind="ExternalInput").ap()
    woT = nc.dram_tensor("woT", [128, D], f32r, kind="ExternalInput").ap()
    P128 = nc.dram_tensor("P128", [128, 128], f32r, kind="ExternalInput").ap()
    I128 = nc.dram_tensor("I128", [128, 128], f32, kind="ExternalInput").ap()
    maskg = nc.dram_tensor(
        "maskg", [128, 128 * max(n_gen, 1)], f32, kind="ExternalInput"
    ).ap()
    ones2 = nc.dram_tensor("ones2", [128, 130], bf16, kind="ExternalInput").ap()
    y = nc.dram_tensor("y", [s, D], bf16, kind="ExternalOutput").ap()

    r = lambda ap: ap

    with tile.TileContext(nc) as tc, ExitStack() as ctx:
        consts = ctx.enter_context(tc.tile_pool(name="consts", bufs=1))
        persist = ctx.enter_context(tc.tile_pool(name="persist", bufs=1))
        xt_pool = ctx.enter_context(tc.tile_pool(name="xt", bufs=24))
        wk_pool = ctx.enter_context(tc.tile_pool(name="work", bufs=4))
        probs_pool = ctx.enter_context(tc.tile_pool(name="probs", bufs=8))
        mask_pool = ctx.enter_context(tc.tile_pool(name="maskb", bufs=2))
        atu_pool = ctx.enter_context(tc.tile_pool(name="atu", bufs=4))
        attn_pool = ctx.enter_context(tc.tile_pool(name="attn", bufs=2))
        bc_pool = ctx.enter_context(tc.tile_pool(name="bc", bufs=4))
        yo_pool = ctx.enter_context(tc.tile_pool(name="yo", bufs=3))
        psum = ctx.enter_context(tc.tile_pool(name="psum", bufs=2, space="PSUM"))

        # ---- constants ----
        # DMA throughput is packet-rate bound (one packet per partition
        # row), so big constants are split by partition halves across
        # otherwise-idle engine queues to parallelize descriptor issue.
        c_wq = consts.tile([128, D], bf16)
        nc.sync.dma_start(c_wq[0:64, :], wqT[0:64, :])
        nc.scalar.dma_start(c_wq[64:128, :], wqT[64:128, :])
        c_wk = consts.tile([128, D], bf16)
        nc.sync.dma_start(c_wk[0:64, :], wkT[0:64, :])
        nc.scalar.dma_start(c_wk[64:128, :], wkT[64:128, :])
        c_wv = consts.tile([128, D], bf16)
        nc.sync.dma_start(c_wv[0:64, :], wvT[0:64, :])
        nc.scalar.dma_start(c_wv[64:128, :], wvT[64:128, :])
        c_P = consts.tile([128, 128], f32r)
        nc.gpsimd.dma_start(c_P[:], P128[:])
        c_I = consts.tile([128, 128], f32)
        nc.gpsimd.dma_start(c_I[:], I128[:])
        # cos/sin are streamed per s-block inside proj(); wo/ones after proj(0)
        c_cos = consts.tile([128, s], f32)
        c_sin = consts.tile([128, s], f32)
        c_one2 = consts.tile([128, 130], bf16)  # cols 0:2 ones, 2:130 zeros
        c_wo = consts.tile([128, D], f32r)

        # pre-load the exp table set while projections run
        warm = wk_pool.tile([1, 2], f32, tag="warm", bufs=1)
        nc.scalar.activation(warm[:], c_I[0:1, 0:2], Exp)

        # ---- persistent activations ----
        qT2 = persist.tile([128, s], bf16)  # [2*64 head rows, s] rope'd & scaled
        kT2 = persist.tile([128, s], bf16)
        v_sb = persist.tile([128, nb * 130], bf16)  # per sk-chunk: [A 64|1|B 64|1]

        # ---- phase-1 worker: q/k/v projections + rope for one 512 s-block ----
        # Returns a list of closures ("steps") so the main loop can weave
        # them between attention chunks: each engine executes its FIFO in
        # emission order, so projection matmuls emitted between score
        # matmuls fill the PE while ScalarE crunches exp.
        def proj_steps(sb):
            ssl = slice(512 * sb, 512 * (sb + 1))
            st = {}

            def s_head():
                # issue every DMA for this block up front: they ride the
                # sync/gpsimd/scalar queues, so by the time the woven matmul
                # steps reach the PE FIFO the tiles have landed
                nc.scalar.dma_start(c_cos[:, ssl], cosT[:, ssl])
                nc.gpsimd.dma_start(c_sin[:, ssl], sinTS[:, ssl])
                st["xts"] = []
                for dc in range(nd):
                    xt = xt_pool.tile([128, 512], bf16)
                    if sb == 0:
                        eng = (nc.sync, nc.gpsimd, nc.scalar)[dc % 3]
                    else:
                        eng = nc.sync if dc % 2 == 0 else nc.gpsimd
                    eng.dma_start(xt[:], xT[128 * dc : 128 * (dc + 1), ssl])
                    st["xts"].append(xt)
                st["psq"] = psum.tile([128, 512], f32, tag="aux", name="psq")
                st["psk"] = psum.tile([128, 512], f32, tag="aux", name="psk")

            def s_qk(dc):
                def f():
                    xt = st["xts"][dc]
                    first, last = dc == 0, dc == nd - 1
                    nc.tensor.matmul(
                        st["psq"][:], r(c_wq[:, 128 * dc : 128 * (dc + 1)]),
                        r(xt[:]), start=first, stop=last, skip_group_check=True,
                    )
                    nc.tensor.matmul(
                        st["psk"][:], r(c_wk[:, 128 * dc : 128 * (dc + 1)]),
                        r(xt[:]), start=first, stop=last, skip_group_check=True,
                    )
                return f

            def s_rcopy(which):
                # DVE part of rope, emitted well before the matmul that
                # consumes `raw` so the PE FIFO never waits on the copy
                def f():
                    ps = st["psq"] if which == "q" else st["psk"]
                    raw = wk_pool.tile([128, 512], f32r, tag="rope", name="raw")
                    nc.vector.tensor_copy(raw[:], ps[:])
                    st["raw" + which] = raw
                return f

            def s_rope(which):
                def f():
                    dst = qT2 if which == "q" else kT2
                    raw = st["raw" + which]
                    psw = psum.tile([128, 512], f32, tag="aux")
                    nc.tensor.matmul(
                        psw[:], r(c_P[:]), r(raw[:]), start=True, stop=True
                    )
                    t1 = wk_pool.tile([128, 512], f32, tag="rope")
                    nc.vector.tensor_mul(t1[:], raw[:], c_cos[:, ssl])
                    t2 = wk_pool.tile([128, 512], f32, tag="rope")
                    nc.vector.tensor_mul(t2[:], psw[:], c_sin[:, ssl])
                    nc.vector.tensor_add(dst[:, ssl], t1[:], t2[:])
                return f

            def s_vhead():
                st["psv"] = psum.tile([128, 512], f32, tag="aux", name="psv")

            def s_v(dc):
                def f():
                    nc.tensor.matmul(
                        st["psv"][:], r(c_wv[:, 128 * dc : 128 * (dc + 1)]),
                        r(st["xts"][dc][:]), start=dc == 0, stop=dc == nd - 1,
                        skip_group_check=True,
                    )
                return f

            def s_vtt():
                vtt = wk_pool.tile([128, 512], f32, tag="vtt", bufs=2)
                nc.vector.tensor_copy(vtt[:], st["psv"][:])
                st["vtt"] = vtt

            def s_tr(k4):
                def f():
                    sc_ = 4 * sb + k4
                    pst = psum.tile([128, 512], f32, tag="aux")
                    nc.tensor.transpose(
                        pst[:, 0:128], st["vtt"][:, 128 * k4 : 128 * (k4 + 1)],
                        c_I[:],
                    )
                    nc.vector.tensor_copy(
                        v_sb[:, 130 * sc_ : 130 * sc_ + 64], pst[:, 0:64]
                    )
                    nc.vector.tensor_copy(
                        v_sb[:, 130 * sc_ + 65 : 130 * sc_ + 129], pst[:, 64:128]
                    )
                return f

            steps = [s_head]
            steps += [s_qk(dc) for dc in range(nd)]
            steps += [s_rcopy("q"), s_vhead, s_v(0), s_v(1), s_rcopy("k")]
            steps += [s_v(2), s_v(3), s_rope("q")]
            steps += [s_v(dc) for dc in range(4, nd)]
            steps += [s_rope("k"), s_vtt]
            steps += [s_tr(k4) for k4 in range(4)]
            return steps

        def proj(sb):
            for f in proj_steps(sb):
                f()

        def kept_for(J):
            kept = []
            for i in range(nb):
                subs = [int(cls[i, 4 * J + u]) for u in range(4)]
                if any(c != 1 for c in subs):
                    kept.append((i, subs))
            return kept

        # ---- attention core for one q block: scores -> exp -> p@v ----
        # leaves unnormalized [65, 512] accumulators (+den in row 64) in SBUF.
        # `fillers` are closures (projection / epilogue steps) emitted between
        # chunks so the PE FIFO always has ready work while exp runs; the p@v
        # matmuls are emitted one chunk late so they never head-of-line block.
        def attn_core(J, kept, fillers=()):
            jsl = slice(512 * J, 512 * (J + 1))
            # batched DMA of the general (mixed) mask blocks for this J
            gen = [(i, u) for i, subs in kept for u in range(4) if subs[u] == 2]
            mb_slc = {}
            if gen:
                gis = sorted(GEN_INDEX[(i, 4 * J + u)] for i, u in gen)
                runs = []
                for g in gis:
                    if runs and g == runs[-1][1]:
                        runs[-1][1] = g + 1
                    else:
                        runs.append([g, g + 1])
                for g0, g1 in runs:
                    mb = mask_pool.tile([128, 512], f32)
                    nc.gpsimd.dma_start(
                        mb[:, : 128 * (g1 - g0)],
                        maskg[:, 128 * g0 : 128 * g1],
                    )
                    for g in range(g0, g1):
                        mb_slc[g] = mb[:, 128 * (g - g0) : 128 * (g - g0 + 1)]
            pvA = psum.tile([128, 512], f32, tag="pv")
            pvB = psum.tile([128, 512], f32, tag="pv")
            n = len(kept)
            fillers = list(fillers)
            fi = 0

            def emit_pv(idx):
                i, subs = kept[idx]
                fk = min(u for u in range(4) if subs[u] != 1)
                pb = pbs[idx]
                first, last = idx == 0, idx == n - 1
                tsl = slice(128 * fk, 512)
                nc.tensor.matmul(
                    pvA[0:65, tsl], r(v_sb[:, 130 * i : 130 * i + 65]),
                    r(pb[:, 128 * fk : 512]),
                    start=first, stop=last, skip_group_check=True,
                )
                nc.tensor.matmul(
                    pvB[0:65, tsl], r(v_sb[:, 130 * i + 65 : 130 * i + 130]),
                    r(pb[:, 512 + 128 * fk : 1024]),
                    start=first, stop=last, skip_group_check=True,
                )

            pbs = []
            for idx, (i, subs) in enumerate(kept):
                isl = slice(128 * i, 128 * (i + 1))
                fk = min(u for u in range(4) if subs[u] != 1)
                sc = psum.tile([128, 1024], f32, tag="sc")
                nc.tensor.matmul(
                    sc[:, 0:512], r(kT2[0:64, isl]), r(qT2[0:64, jsl]),
                    start=True, stop=True, tile_position=(0, 0),
                )
                nc.tensor.matmul(
                    sc[:, 512:1024], r(kT2[64:128, isl]), r(qT2[64:128, jsl]),
                    start=True, stop=True, tile_position=(64, 0),
                )
                sc3v = sc[:].rearrange("p (h q) -> p h q", h=2)
                for u, cu in enumerate(subs):
                    if cu == 2:
                        mbs = mb_slc[GEN_INDEX[(i, 4 * J + u)]]
                        s3 = sc3v[:, :, 128 * u : 128 * (u + 1)]
                        nc.vector.tensor_add(
                            s3, s3, mbs[:, None, :].broadcast_to([128, 2, 128])
                        )
                pb = probs_pool.tile([128, 1024], bf16)
                pbs.append(pb)
                if fk:
                    sc3 = sc[:].rearrange("p (h q) -> p h q", h=2)[:, :, 128 * fk : 512]
                    pb3 = pb[:].rearrange("p (h q) -> p h q", h=2)[:, :, 128 * fk : 512]
                    nc.scalar.activation(pb3, sc3, Exp, bias=-neg_bias)
                else:
                    nc.scalar.activation(pb[:], sc[:], Exp, bias=-neg_bias)
                # middle fully-masked sub-blocks (never happens for causal):
                for u, cu in enumerate(subs):
                    if cu == 1 and u > fk:
                        for off in (0, 512):
                            usl = slice(off + 128 * u, off + 128 * (u + 1))
                            nc.vector.tensor_copy(pb[:, usl], c_one2[:, 2:130])
                # weave in filler work, then the previous chunk's p@v
                want = (idx + 1) * len(fillers) // n
                while fi < want:
                    fillers[fi]()
                    fi += 1
                if idx > 0:
                    emit_pv(idx - 1)
            while fi < len(fillers):
                fillers[fi]()
                fi += 1
            emit_pv(n - 1)
            # evacuate unnormalized accumulators; releases the pv PSUM slots
            atus = []
            for pv in (pvA, pvB):
                atu = atu_pool.tile([65, 512], f32)
                nc.vector.tensor_copy(atu[:], pv[0:65, :])
                atus.append(atu)
            return atus

        # ---- epilogue for one q block: normalize, project, store ----
        # Split into [head..., tail...] step closures: the reciprocal chain
        # (DVE/DMA/gpsimd, long latency, no PE) goes early in the weave, the
        # output-projection matmuls late (by then `at` is ready).
        def epilogue_steps(J, atus):
            atuA, atuB = atus
            st = {"bcs": []}

            def e_den(atu):
                def f():
                    d0 = wk_pool.tile([1, 512], f32, tag="d0", bufs=8)
                    nc.gpsimd.dma_start(d0[:], atu[64:65, :])
                    r0 = wk_pool.tile([1, 512], f32, tag="d0", bufs=8)
                    s0 = wk_pool.tile([1, 512], f32, tag="d0", bufs=8)
                    nc.vector.reciprocal_approx_accurate(r0[:], d0[:], s0[:])
                    bc = bc_pool.tile([64, 512], f32)
                    nc.gpsimd.partition_broadcast(bc[:], r0[:])
                    st["bcs"].append(bc)
                return f

            def e_at():
                at = attn_pool.tile([128, 512], f32r)
                atBt = attn_pool.tile([64, 512], f32r, tag="atB", bufs=2)
                nc.vector.tensor_mul(at[0:64, :], atuA[0:64, :], st["bcs"][0][:])
                nc.vector.tensor_mul(atBt[:], atuB[0:64, :], st["bcs"][1][:])
                nc.gpsimd.dma_start(at[64:128, :], atBt[:])
                st["at"] = at

            def e_out(scn):
                def f():
                    at = st["at"]
                    ssl2 = slice(128 * scn, 128 * (scn + 1))
                    rows = slice(512 * J + 128 * scn, 512 * J + 128 * (scn + 1))
                    yo = yo_pool.tile([128, 1024], bf16)
                    for do2 in range(2):
                        dsl = slice(512 * do2, 512 * (do2 + 1))
                        pso = psum.tile([128, 512], f32, tag="aux")
                        nc.tensor.matmul(
                            pso[:], r(at[:, ssl2]), r(c_wo[:, dsl]),
                            start=True, stop=True,
                        )
                        nc.vector.tensor_copy(yo[:, dsl], pso[:])
                    nc.sync.dma_start(y[rows, :], yo[:])
                return f

            head = [e_den(atuA), e_den(atuB), e_at]
            tail = [e_out(scn) for scn in range(4)]
            return head, tail

        def epilogue(J, atus):
            head, tail = epilogue_steps(J, atus)
            for f in head + tail:
                f()

        # ---- main schedule: 1-block software pipeline ----
        # iteration J: [epilogue J-1 dataflow] + [attn core J] + [proj J+1]
        kept_all = [kept_for(J) for J in range(NJ)]
        done_sb = 0
        pending = None  # (J, atus) awaiting epilogue
        first = True
        for J in range(NJ):
            need_sb = J
            if kept_all[J]:
                need_sb = max(need_sb, max(i for i, _ in kept_all[J]) // 4)
            while done_sb <= need_sb:
                proj(done_sb)
                done_sb += 1
            if first:
                # late-needed constants, deferred so they don't crowd the
                # startup DMA stream feeding the first projections
                nc.gpsimd.dma_start(c_one2[:], ones2[:])
                nc.gpsimd.dma_start(c_wo[:], woT[:])
                # ones columns of v_aug (cols 64/129 of each 130-wide chunk)
                ones_view = v_sb[:].rearrange("p (c w) -> p c w", w=130)[
                    :, :, 64:130:65
                ]
                nc.vector.tensor_copy(
                    ones_view, c_one2[:, None, 0:2].broadcast_to([128, nb, 2])
                )
                first = False
            fillers = []
            late = []
            if pending is not None:
                head, tail = epilogue_steps(*pending)
                fillers += head
                late = tail
                pending = None
            if J + 1 < NJ and done_sb == J + 1 and kept_all[J]:
                fillers += proj_steps(done_sb)
                done_sb += 1
            fillers += late
            if kept_all[J]:
                atus = attn_core(J, kept_all[J], fillers)
                pending = (J, atus)
            else:
                for f in fillers:
                    f()
        if pending is not None:
            epilogue(*pending)

    nc.compile()
    return nc


GEN_INDEX = {}


def host_prep(x, freqs_cos, freqs_sin, mask, wq, wk, wv, wo):
    """Build per-core input maps + mask classification.  Returns
    (in_maps, cls, n_gen, neg_bias)."""
    global GEN_INDEX
    s = x.shape[1]
    import ml_dtypes

    x2 = np.ascontiguousarray(x.reshape(s, D))
    xT = np.ascontiguousarray(x2.T).astype(ml_dtypes.bfloat16)

    # rope tables in T layout (same for q and k; q scale folded into wq)
    p = np.arange(128)
    j = (p % HD) // 2  # freq index per partition row
    cosT = np.ascontiguousarray(freqs_cos.T[j, :]).astype(np.float32)  # [128, s]
    sinT = freqs_sin.T[j, :]
    sign = np.where(p % 2 == 0, -1.0, 1.0).astype(np.float32)
    sinTS = np.ascontiguousarray(sinT * sign[:, None]).astype(np.float32)

    # swap-adjacent permutation and identity
    P = np.zeros((128, 128), np.float32)
    P[np.arange(128) ^ 1, np.arange(128)] = 1.0
    I = np.eye(128, dtype=np.float32)

    # mask classification + general block packing
    maskT = np.ascontiguousarray(mask.T).astype(np.float32)
    cls = classify_mask(maskT, s)
    GEN_INDEX = {}
    gen_blocks = []
    nbk = s // 128
    for i in range(nbk):
        for jj in range(nbk):
            if cls[i, jj] == 2:
                GEN_INDEX[(i, jj)] = len(gen_blocks)
                gen_blocks.append(
                    maskT[128 * i : 128 * (i + 1), 128 * jj : 128 * (jj + 1)]
                )
    n_gen = len(gen_blocks)
    if n_gen:
        maskg = np.ascontiguousarray(np.concatenate(gen_blocks, axis=1))
    else:
        maskg = np.zeros((128, 128), np.float32)

    # softmax shift: cheap norm bound first (rope preserves pair norms);
    # only fall back to the exact per-head max if the bound exceeds 60.
    qf = x2 @ wq.T
    kf = x2 @ wk.T
    qn = kn = 0.0
    for h in range(H):
        hs = slice(HD * h, HD * (h + 1))
        qn = max(qn, float(np.linalg.norm(qf[:, hs], axis=1).max()))
        kn = max(kn, float(np.linalg.norm(kf[:, hs], axis=1).max()))
    m_bound = qn * kn / 8.0
    if m_bound <= 60.0:
        neg_bias = 0.0
    else:
        qh = _rope_np(qf, freqs_cos, freqs_sin)
        kh = _rope_np(kf, freqs_cos, freqs_sin)
        m = 0.0
        for h in range(H):
            qs = qh[:, HD * h : HD * (h + 1)]
            ks = kh[:, HD * h : HD * (h + 1)]
            m = max(m, float(np.abs(qs @ ks.T).max()) / 8.0)
        neg_bias = max(0.0, m - 60.0)

    def pack_w(w_slice):  # [EC, D] -> [128, D] chunked-transpose layout
        t = np.ascontiguousarray(w_slice.T)  # [D, EC=128]
        return np.ascontiguousarray(
            t.reshape(D // 128, 128, 128).transpose(1, 0, 2).reshape(128, D)
        ).astype(ml_dtypes.bfloat16)

    ones130 = np.zeros((128, 130), ml_dtypes.bfloat16)
    ones130[:, 0:2] = 1.0
    in_maps = []
    for c in range(NCORES):
        R = slice(EC * c, EC * (c + 1))
        woT_pack = np.ascontiguousarray(wo[:, R].T).astype(np.float32)  # [128, D]
        in_maps.append(
            {
                "xT": xT,
                "cosT": cosT,
                "sinTS": sinTS,
                "wqT": pack_w(wq[R] * 0.125),
                "wkT": pack_w(wk[R]),
                "wvT": pack_w(wv[R]),
                "woT": woT_pack,
                "P128": P,
                "I128": I,
                "maskg": maskg,
                "ones2": ones130,
            }
        )
    return in_maps, cls, n_gen, neg_bias


def _rope_np(t, cos, sin):
    s = t.shape[0]
    tr = t.reshape(s, H, HD // 2, 2)
    te, to = tr[..., 0], tr[..., 1]
    c = cos[:, None, :]
    sn = sin[:, None, :]
    oe = te * c - to * sn
    oo = te * sn + to * c
    return np.stack([oe, oo], axis=-1).reshape(s, H * HD)


def kernel(**inputs):
    from concourse.bass_utils import run_bass_kernel_spmd

    x = np.asarray(inputs["x"], np.float32)
    in_maps, cls, n_gen, neg_bias = host_prep(
        x,
        np.asarray(inputs["freqs_cos"], np.float32),
        np.asarray(inputs["freqs_sin"], np.float32),
        np.asarray(inputs["mask"], np.float32),
        np.asarray(inputs["wq"], np.float32),
        np.asarray(inputs["wk"], np.float32),
        np.asarray(inputs["wv"], np.float32),
        np.asarray(inputs["wo"], np.float32),
    )
    key = (x.shape[1], cls.tobytes(), n_gen, float(neg_bias))
    if key not in _PROGRAM_CACHE:
        _PROGRAM_CACHE[key] = build_program(x.shape[1], cls, n_gen, neg_bias)
    nc = _PROGRAM_CACHE[key]
    res = run_bass_kernel_spmd(nc, in_maps, core_ids=list(range(NCORES)))
    y = np.zeros((x.shape[1], D), np.float32)
    for c in range(NCORES):
        y += np.asarray(res.results[c]["y"], np.float32)
    return y.reshape(x.shape)
